# revision 22
# baseline (speedup 1.0000x reference)
"""Trainium2 Bass kernel for nn_AttentionFusion (8-core data-parallel over B).

Reference computation per batch b:
    p_proj = X @ W_p + b_p                      # (N, C)
    c_proj = CF @ W_c + b_c                     # (NC, C)
    S      = p_proj @ c_proj.T                  # (N, NC)
    W      = softmax(S, axis=-1)
    out    = X + W @ CF                         # (N, C)

Algebraic refactor (exact in real arithmetic):
    S = X @ M + 1·t  with  M = W_p @ c_proj.T (C×NC),  t = b_p @ c_proj.T (NC)
so the (N,C)x(C,C) projection matmul disappears; per-core PE work is the X
transposes (scores need C on partitions), the scores matmul, and the
weighted sum.

v2 changes vs the fp32 baseline (tolerance is 2e-2; fp22 everywhere in the
scores path costs ~2e-4):
  * transposes and scores matmuls run f32r (1.5 / 1.0 cyc per moving row
    instead of 2.0 / 4.0) -> PE drops from ~124us to ~75us busy.
  * no M duplication and no fp22-compensation rows: S^T/exp are [64, 512],
    the weighted matmul contracts K=64 against [fp22(CF) | 1].
  * PE is software-pipelined one half-tile: the weighted matmuls of half
    h-1 are emitted after the scores of half h so PE never stalls on exp.
  * epilogue: normalize alternates ACT/DVE into a [P,2,C] tmp, residual
    adds run on GPSIMD as two fused [128,512] tensor_tensor per half.
  * output is stored per half (512 rows, 4KiB/partition contiguous runs).

Sharding: B=8 batches -> one batch per NeuronCore, weights replicated.
"""

import numpy as np

B, N, NC, C = 8, 16384, 64, 256
P = 128  # SBUF partitions
SUPER_ROWS = 1024  # rows per DMA supertile (row = s*1024 + p*8 + j)
JCHUNK = SUPER_ROWS // P  # 8 row-chunks per supertile
HALF = 4  # chunks per scores tile (4*128 = 512 rows)
NSUPER = N // SUPER_ROWS

_CACHE = {}


def _split_multiwait_ctrl(nc, mybir):
    """This toolchain's walrus accepts only ONE sync wait per instruction,
    but Tile's scheduler attaches one wait per depended-on proc.  Keep the
    last wait on the instruction and hoist the excess onto single-wait NoOps
    inserted immediately before it on the same engine (same sequencer order,
    identical blocking semantics)."""
    for f in nc.m.functions:
        for bb in f.blocks:
            insts = bb.instructions
            new_list = []
            changed = False
            for inst in insts:
                si = inst.sync_info
                if si is not None and si.on_wait and len(si.on_wait) > 1:
                    waits = list(si.on_wait)
                    for w in waits[:-1]:
                        nop = mybir.InstNoOp(
                            name=nc.get_next_instruction_name(),
                            engine=inst.engine,
                            sync_info=mybir.SyncInfo(on_wait=[w], on_update=[]),
                            bass_nofuse=True,
                        )
                        nc.register_instruction(nop, overwrite=True)
                        new_list.append(nop)
                        changed = True
                    inst.sync_info = mybir.SyncInfo(
                        on_wait=[waits[-1]], on_update=list(si.on_update or [])
                    )
                new_list.append(inst)
            if changed:
                bb.instructions[:] = new_list
    return nc


def _build():
    from contextlib import ExitStack

    import concourse.bass as bass
    import concourse.mybir as mybir
    import concourse.tile as tile
    from concourse.masks import make_identity

    f32 = mybir.dt.float32
    f32r = mybir.dt.float32r
    Exp = mybir.ActivationFunctionType.Exp

    nc = bass.Bass("TRN2", target_bir_lowering=False, debug=False)
    x = nc.declare_dram_parameter("x", [N, C], f32, isOutput=False)
    cf = nc.declare_dram_parameter("cf", [NC, C], f32, isOutput=False)
    wp = nc.declare_dram_parameter("wp", [C, C], f32, isOutput=False)
    bp = nc.declare_dram_parameter("bp", [C], f32, isOutput=False)
    wc = nc.declare_dram_parameter("wc", [C, C], f32, isOutput=False)
    bc = nc.declare_dram_parameter("bc", [C], f32, isOutput=False)
    out = nc.declare_dram_parameter("out", [N, C], f32, isOutput=True)

    KC = C // P  # 2 contraction chunks of 128 over the C dim
    RW = HALF * P  # 512 rows per scores tile

    with tile.TileContext(nc) as tc:
        with (
            tc.tile_pool(name="const", bufs=1) as const,
            tc.tile_pool(name="xin", bufs=6) as xin,
            tc.tile_pool(name="oout", bufs=4) as oout,
            tc.tile_pool(name="work", bufs=5) as work,
        ):
            setup_stack = ExitStack()
            setup_ps = setup_stack.enter_context(
                tc.tile_pool(name="setup_ps", bufs=2, space="PSUM")
            )
            # ---------------- setup: identity, weights, M, t, cfstack ------
            x_view = x.rearrange("(s p j) c -> s p j c", p=P, j=JCHUNK)
            ident = const.tile([P, P], f32)
            make_identity(nc, ident)
            # dummy activation: hoists the ACT exp-table load off the
            # critical path (it otherwise fires right before the first exp)
            warm = const.tile([1, 2], f32)
            nc.vector.memset(warm, 0.0)
            nc.scalar.activation(warm, warm, Exp)

            # weight loads go FIRST on the Sync queue: the queue is in-order,
            # so the tiny weight transfers (0.6 MB) land before the x-load
            # flood and setup compute (which gates all in-order PE work)
            # finishes early
            cf_sb = const.tile([NC, C], f32)
            nc.sync.dma_start(cf_sb, cf.ap())
            bp_sb = const.tile([P, KC], f32)
            nc.sync.dma_start(bp_sb, bp.rearrange("(o p) -> p o", p=P))
            bc_sb = const.tile([P, KC], f32)
            nc.sync.dma_start(bc_sb, bc.rearrange("(o p) -> p o", p=P))
            wc_sb = const.tile([P, KC, C], f32)
            nc.sync.dma_start(wc_sb, wc.rearrange("(o p) d -> p o d", p=P))
            wp_sb = const.tile([P, KC, C], f32)
            nc.sync.dma_start(wp_sb, wp.rearrange("(o p) d -> p o d", p=P))

            # prime the input pipeline: 5 supertile loads queued up front
            # (6 bufs: the rolling load then reuses a buffer whose consumers
            # finished a full supertile earlier, so its trigger never waits)
            x_tiles = {}

            def load_x(sp):
                t = xin.tile([P, JCHUNK, C], f32, tag="x_tile", name="x_tile")
                nc.sync.dma_start(t, x_view[sp])
                x_tiles[sp] = t

            for sp in range(min(5, NSUPER)):
                load_x(sp)

            # cfT[c, k] = CF[k, c]   as [128, KC, NC]
            cfT = const.tile([P, KC, NC], f32)
            for i in range(KC):
                pt = setup_ps.tile([P, NC], f32, tag="setup")
                nc.tensor.transpose(pt, cf_sb[:, bass.ts(i, P)], ident[:NC, :NC])
                if i == 0:
                    nc.vector.tensor_copy(cfT[:, i, :], pt)
                else:
                    nc.scalar.copy(cfT[:, i, :], pt)

            # c_projT[d, k] = sum_c W_c[c,d] cfT[c,k] + b_c[d]   as [128, KC, NC]
            cprojT = const.tile([P, KC, NC], f32)
            for i in range(KC):
                pt = setup_ps.tile([P, NC], f32, tag="setup")
                for k in range(KC):
                    nc.tensor.matmul(
                        pt,
                        wc_sb[:, k, bass.ts(i, P)],
                        cfT[:, k, :],
                        start=(k == 0),
                        stop=(k == KC - 1),
                    )
                if i == 0:
                    nc.vector.tensor_scalar_add(
                        cprojT[:, i, :], pt, bc_sb[:, i : i + 1]
                    )
                else:
                    nc.scalar.add(cprojT[:, i, :], pt, bc_sb[:, i : i + 1])

            # wpT[d, c] = W_p[c, d]   as [128, KC, C]
            wpT = const.tile([P, KC, C], f32)
            for i in range(KC):  # d chunk
                pt = setup_ps.tile([P, KC, P], f32, tag="setupw")
                for j in range(KC):  # c chunk
                    nc.tensor.transpose(pt[:, j, :], wp_sb[:, j, bass.ts(i, P)], ident)
                if i == 0:
                    nc.vector.tensor_copy(wpT[:, i, :], pt)
                else:
                    nc.scalar.copy(wpT[:, i, :], pt)

            # M[c, k] = sum_d W_p[c,d] c_projT[d,k]  as f32r [128, KC, NC]
            mc_sb = const.tile([P, KC, NC], f32r)
            for i in range(KC):  # c chunk
                pt = setup_ps.tile([P, NC], f32, tag="setup")
                for k in range(KC):  # d chunk
                    nc.tensor.matmul(
                        pt,
                        wpT[:, k, bass.ts(i, P)],
                        cprojT[:, k, :],
                        start=(k == 0),
                        stop=(k == KC - 1),
                    )
                if i == 0:
                    nc.vector.tensor_copy(mc_sb[:, i, :], pt)
                else:
                    nc.scalar.copy(mc_sb[:, i, :], pt)

            # tT[k] = sum_d c_projT[d,k] b_p[d]   as [NC, 1] (exp bias)
            t_ps = setup_ps.tile([NC, 1], f32, tag="setup_t")
            for k in range(KC):
                nc.tensor.matmul(
                    t_ps,
                    cprojT[:, k, :],
                    bp_sb[:, k : k + 1],
                    start=(k == 0),
                    stop=(k == KC - 1),
                )
            tT = const.tile([NC, 1], f32)
            nc.vector.tensor_copy(tT, t_ps)

            # cfstack [64, C+2] f32r = [fp22(CF) | 1 | 1]: one weighted-sum
            # matmul yields the weighted sum and the softmax normalizer.
            # (f32r matmuls need EVEN moving/dst free sizes, hence C+2.)
            cfstack = const.tile([NC, C + 2], f32r)
            nc.vector.tensor_copy(cfstack[:, :C], cf_sb)
            ones1 = const.tile([NC, 2], f32)
            nc.vector.memset(ones1, 1.0)
            nc.vector.tensor_copy(cfstack[:, C : C + 2], ones1)

            # ---------------- main loop --------------------------------------
            setup_stack.close()
            ps_stack = ExitStack()
            ps_xt = ps_stack.enter_context(
                tc.tile_pool(name="ps_xt", bufs=1, space="PSUM")
            )
            ps_sc = ps_stack.enter_context(
                tc.tile_pool(name="ps_sc", bufs=2, space="PSUM")
            )
            ps_ws = ps_stack.enter_context(
                tc.tile_pool(name="ps_ws", bufs=1, space="PSUM")
            )
            o_view = out.rearrange("(s p j) c -> s p j c", p=P, j=JCHUNK)

            def epilogue(st):
                """Weighted matmuls + normalize + residual for a finished
                half (one stage behind the scores pipeline)."""
                s, h, expT, x_tile, o_tile = st
                # one PSUM tile spanning 4 banks: chunk jj's weighted sum
                # lands in bank jj (512-f32 pitch), so the 4 softmax
                # normalizers live at a clean stride and ONE reciprocal
                # covers the whole half
                ws_all = ps_ws.tile([P, HALF, 512], f32, tag="ws", name="ws_all")
                for jj in range(HALF):
                    nc.tensor.matmul(
                        ws_all[:, jj, : C + 2],
                        expT[:, bass.ts(jj, P)],
                        cfstack,
                        start=True,
                        stop=True,
                    )
                recip_all = work.tile([P, HALF], f32, tag="recip")
                nc.vector.reciprocal(recip_all, ws_all[:, :, C])
                tmp = work.tile([P, HALF, C], f32, tag="tmp")
                for jj in range(HALF):
                    if jj % 2 == 0:
                        nc.scalar.mul(
                            tmp[:, jj, :], ws_all[:, jj, :C], recip_all[:, jj : jj + 1]
                        )
                    else:
                        nc.vector.tensor_scalar_mul(
                            tmp[:, jj, :], ws_all[:, jj, :C], recip_all[:, jj : jj + 1]
                        )
                j0 = h * HALF
                tail = s >= NSUPER - 2
                if tail and h == 1:
                    # drain the GPSIMD backlog in parallel at the tail: DVE
                    # is idle once the last muls retire
                    nc.vector.tensor_add(
                        o_tile[:, j0 : j0 + HALF, :], tmp, x_tile[:, j0 : j0 + HALF, :]
                    )
                else:
                    nc.gpsimd.tensor_add(
                        o_tile[:, j0 : j0 + HALF, :], tmp, x_tile[:, j0 : j0 + HALF, :]
                    )
                # stores ride the GpSimd DGE queue: GPSIMD's adds are the
                # store's data dependency, so the wait is already resolved
                # when its sequencer reaches the dma_start (any other engine
                # would block its pipeline on the wait).  The tail stores go
                # on the idle Sync queue instead (all loads are done), and
                # the last supertile stores per half for a shorter drain.
                if s == NSUPER - 1:
                    nc.sync.dma_start(
                        o_view[s, :, h * HALF : (h + 1) * HALF],
                        o_tile[:, h * HALF : (h + 1) * HALF],
                    )
                elif h == 1:
                    if tail:
                        nc.sync.dma_start(o_view[s], o_tile)
                    else:
                        nc.gpsimd.dma_start(o_view[s], o_tile)

            prev = None
            for s in range(NSUPER):
                x_tile = x_tiles.pop(s)
                o_tile = oout.tile([P, JCHUNK, C], f32)

                for h in range(JCHUNK // HALF):
                    # X^T for 512 rows: per c-chunk k, [128, 512] (free =
                    # jj*128 + p  <->  row s*1024 + p*8 + (h*HALF+jj))
                    xt_ps = [
                        ps_xt.tile([P, RW], f32, tag=f"xt{k}", name=f"xt_ps{k}")
                        for k in range(KC)
                    ]
                    for jj in range(HALF):
                        j = h * HALF + jj
                        for k in range(KC):
                            nc.tensor.transpose(
                                xt_ps[k][:, bass.ts(jj, P)],
                                x_tile[:, j, bass.ts(k, P)],
                                ident,
                            )
                    xt_sb = [
                        work.tile([P, RW], f32r, tag=f"xt_sb{k}", name=f"xt_sb{k}")
                        for k in range(KC)
                    ]
                    # alternate the PSUM->SBUF copies between DVE and ACT
                    nc.vector.tensor_copy(xt_sb[0], xt_ps[0])
                    nc.scalar.copy(xt_sb[1], xt_ps[1])

                    # S^T[k, r] = sum_c M[c,k] X[r,c]
                    sc_ps = ps_sc.tile([NC, RW], f32, tag="sc")
                    for k in range(KC):
                        nc.tensor.matmul(
                            sc_ps,
                            mc_sb[:, k, :],
                            xt_sb[k],
                            start=(k == 0),
                            stop=(k == KC - 1),
                        )

                    # expT = exp(S^T + t)  (f32r: feeds the f32r matmul)
                    expT = work.tile([NC, RW], f32r, tag="expT")
                    nc.scalar.activation(expT, sc_ps, Exp, bias=tT)

                    if prev is not None:
                        epilogue(prev)
                        if h == 0 and s + 4 < NSUPER:
                            # rolling prefetch: the buffer being reused held
                            # x(s-1), whose last reader (epilogue (s-1,1))
                            # was just emitted above
                            load_x(s + 4)
                    prev = (s, h, expT, x_tile, o_tile)

            epilogue(prev)
            ps_stack.close()

    return _split_multiwait_ctrl(nc, mybir)


def _get_nc():
    if "nc" not in _CACHE:
        _CACHE["nc"] = _build()
    return _CACHE["nc"]


def run(inputs, trace=False):
    from concourse.bass_utils import run_bass_kernel_spmd

    nc = _get_nc()
    pf = np.ascontiguousarray(np.asarray(inputs["point_features"], dtype=np.float32))
    cfeat = np.ascontiguousarray(
        np.asarray(inputs["centroid_features"], dtype=np.float32)
    )
    wp = np.ascontiguousarray(np.asarray(inputs["W_p"], dtype=np.float32))
    bp = np.ascontiguousarray(np.asarray(inputs["b_p"], dtype=np.float32))
    wc = np.ascontiguousarray(np.asarray(inputs["W_c"], dtype=np.float32))
    bc = np.ascontiguousarray(np.asarray(inputs["b_c"], dtype=np.float32))

    in_maps = [
        {"x": pf[b], "cf": cfeat[b], "wp": wp, "bp": bp, "wc": wc, "bc": bc}
        for b in range(B)
    ]
    res = run_bass_kernel_spmd(nc, in_maps, core_ids=list(range(B)), trace=trace)
    out = np.stack([res.results[b]["out"] for b in range(B)], axis=0)
    return out, res


def kernel(**inputs) -> np.ndarray:
    out, _ = run(inputs, trace=False)
    return out


# revision 23
# speedup vs baseline: 1.5253x; 1.5253x over previous
"""Trainium2 Bass kernel for nn_AttentionFusion (8-core data-parallel over B).

Reference computation per batch b:
    p_proj = X @ W_p + b_p                      # (N, C)
    c_proj = CF @ W_c + b_c                     # (NC, C)
    S      = p_proj @ c_proj.T                  # (N, NC)
    W      = softmax(S, axis=-1)
    out    = X + W @ CF                         # (N, C)

Algebraic refactor (exact in real arithmetic):
    S = X @ M + 1·t  with  M = W_p @ c_proj.T (C×NC),  t = b_p @ c_proj.T (NC)
so the (N,C)x(C,C) projection matmul disappears; per-core PE work is the X
transposes (scores need C on partitions), the scores matmul, and the
weighted sum.

v2 changes vs the fp32 baseline (tolerance is 2e-2; fp22 everywhere in the
scores path costs ~2e-4):
  * transposes and scores matmuls run f32r (1.5 / 1.0 cyc per moving row
    instead of 2.0 / 4.0) -> PE drops from ~124us to ~75us busy.
  * no M duplication and no fp22-compensation rows: S^T/exp are [64, 512],
    the weighted matmul contracts K=64 against [fp22(CF) | 1].
  * PE is software-pipelined one half-tile: the weighted matmuls of half
    h-1 are emitted after the scores of half h so PE never stalls on exp.
  * epilogue: normalize alternates ACT/DVE into a [P,2,C] tmp, residual
    adds run on GPSIMD as two fused [128,512] tensor_tensor per half.
  * output is stored per half (512 rows, 4KiB/partition contiguous runs).

Sharding: B=8 batches -> one batch per NeuronCore, weights replicated.
"""

import numpy as np

B, N, NC, C = 8, 16384, 64, 256
P = 128  # SBUF partitions
SUPER_ROWS = 1024  # rows per DMA supertile (row = s*1024 + p*8 + j)
JCHUNK = SUPER_ROWS // P  # 8 row-chunks per supertile
HALF = 4  # chunks per scores tile (4*128 = 512 rows)
NSUPER = N // SUPER_ROWS

_CACHE = {}


def _split_multiwait_ctrl(nc, mybir):
    """This toolchain's walrus accepts only ONE sync wait per instruction,
    but Tile's scheduler attaches one wait per depended-on proc.  Keep the
    last wait on the instruction and hoist the excess onto single-wait NoOps
    inserted immediately before it on the same engine (same sequencer order,
    identical blocking semantics)."""
    for f in nc.m.functions:
        for bb in f.blocks:
            insts = bb.instructions
            new_list = []
            changed = False
            for inst in insts:
                si = inst.sync_info
                if si is not None and si.on_wait and len(si.on_wait) > 1:
                    waits = list(si.on_wait)
                    for w in waits[:-1]:
                        nop = mybir.InstNoOp(
                            name=nc.get_next_instruction_name(),
                            engine=inst.engine,
                            sync_info=mybir.SyncInfo(on_wait=[w], on_update=[]),
                            bass_nofuse=True,
                        )
                        nc.register_instruction(nop, overwrite=True)
                        new_list.append(nop)
                        changed = True
                    inst.sync_info = mybir.SyncInfo(
                        on_wait=[waits[-1]], on_update=list(si.on_update or [])
                    )
                new_list.append(inst)
            if changed:
                bb.instructions[:] = new_list
    return nc


def _build():
    from contextlib import ExitStack

    import concourse.bass as bass
    import concourse.mybir as mybir
    import concourse.tile as tile
    from concourse.masks import make_identity

    f32 = mybir.dt.float32
    f32r = mybir.dt.float32r
    Exp = mybir.ActivationFunctionType.Exp

    nc = bass.Bass("TRN2", target_bir_lowering=False, debug=False)
    x = nc.declare_dram_parameter("x", [N, C], f32, isOutput=False)
    cf = nc.declare_dram_parameter("cf", [NC, C], f32, isOutput=False)
    wp = nc.declare_dram_parameter("wp", [C, C], f32, isOutput=False)
    bp = nc.declare_dram_parameter("bp", [C], f32, isOutput=False)
    wc = nc.declare_dram_parameter("wc", [C, C], f32, isOutput=False)
    bc = nc.declare_dram_parameter("bc", [C], f32, isOutput=False)
    out = nc.declare_dram_parameter("out", [N, C], f32, isOutput=True)

    KC = C // P  # 2 contraction chunks of 128 over the C dim
    RW = HALF * P  # 512 rows per scores tile

    with tile.TileContext(nc) as tc:
        with (
            tc.tile_pool(name="const", bufs=1) as const,
            tc.tile_pool(name="xin", bufs=6) as xin,
            tc.tile_pool(name="oout", bufs=4) as oout,
            tc.tile_pool(name="work", bufs=5) as work,
        ):
            setup_stack = ExitStack()
            setup_ps = setup_stack.enter_context(
                tc.tile_pool(name="setup_ps", bufs=2, space="PSUM")
            )
            # ---------------- setup: identity, weights, M, t, cfstack ------
            x_view = x.rearrange("(s p j) c -> s p j c", p=P, j=JCHUNK)
            ident = const.tile([P, P], f32)
            make_identity(nc, ident)
            # dummy activation: hoists the ACT exp-table load off the
            # critical path (it otherwise fires right before the first exp)
            warm = const.tile([1, 2], f32)
            nc.vector.memset(warm, 0.0)
            nc.scalar.activation(warm, warm, Exp)

            # weight loads go FIRST on the Sync queue: the queue is in-order,
            # so the tiny weight transfers (0.6 MB) land before the x-load
            # flood and setup compute (which gates all in-order PE work)
            # finishes early
            cf_sb = const.tile([NC, C], f32)
            nc.sync.dma_start(cf_sb, cf.ap())
            bp_sb = const.tile([P, KC], f32)
            nc.sync.dma_start(bp_sb, bp.rearrange("(o p) -> p o", p=P))
            bc_sb = const.tile([P, KC], f32)
            nc.sync.dma_start(bc_sb, bc.rearrange("(o p) -> p o", p=P))
            wc_sb = const.tile([P, KC, C], f32)
            nc.sync.dma_start(wc_sb, wc.rearrange("(o p) d -> p o d", p=P))
            wp_sb = const.tile([P, KC, C], f32)
            nc.sync.dma_start(wp_sb, wp.rearrange("(o p) d -> p o d", p=P))

            # prime the input pipeline: 5 supertile loads queued up front
            # (6 bufs: the rolling load then reuses a buffer whose consumers
            # finished a full supertile earlier, so its trigger never waits)
            x_tiles = {}

            def load_x(sp):
                t = xin.tile([P, JCHUNK, C], f32, tag="x_tile", name="x_tile")
                nc.sync.dma_start(t, x_view[sp])
                x_tiles[sp] = t

            for sp in range(min(5, NSUPER)):
                load_x(sp)

            # cfT[c, k] = CF[k, c]   as [128, KC, NC]
            cfT = const.tile([P, KC, NC], f32)
            for i in range(KC):
                pt = setup_ps.tile([P, NC], f32, tag="setup")
                nc.tensor.transpose(pt, cf_sb[:, bass.ts(i, P)], ident[:NC, :NC])
                if i == 0:
                    nc.vector.tensor_copy(cfT[:, i, :], pt)
                else:
                    nc.scalar.copy(cfT[:, i, :], pt)

            # c_projT[d, k] = sum_c W_c[c,d] cfT[c,k] + b_c[d]   as [128, KC, NC]
            cprojT = const.tile([P, KC, NC], f32)
            for i in range(KC):
                pt = setup_ps.tile([P, NC], f32, tag="setup")
                for k in range(KC):
                    nc.tensor.matmul(
                        pt,
                        wc_sb[:, k, bass.ts(i, P)],
                        cfT[:, k, :],
                        start=(k == 0),
                        stop=(k == KC - 1),
                    )
                if i == 0:
                    nc.vector.tensor_scalar_add(
                        cprojT[:, i, :], pt, bc_sb[:, i : i + 1]
                    )
                else:
                    nc.scalar.add(cprojT[:, i, :], pt, bc_sb[:, i : i + 1])

            # wpT[d, c] = W_p[c, d]   as [128, KC, C]
            wpT = const.tile([P, KC, C], f32)
            for i in range(KC):  # d chunk
                pt = setup_ps.tile([P, KC, P], f32, tag="setupw")
                for j in range(KC):  # c chunk
                    nc.tensor.transpose(pt[:, j, :], wp_sb[:, j, bass.ts(i, P)], ident)
                if i == 0:
                    nc.vector.tensor_copy(wpT[:, i, :], pt)
                else:
                    nc.scalar.copy(wpT[:, i, :], pt)

            # M[c, k] = sum_d W_p[c,d] c_projT[d,k]  as f32r [128, KC, NC]
            mc_sb = const.tile([P, KC, NC], f32r)
            for i in range(KC):  # c chunk
                pt = setup_ps.tile([P, NC], f32, tag="setup")
                for k in range(KC):  # d chunk
                    nc.tensor.matmul(
                        pt,
                        wpT[:, k, bass.ts(i, P)],
                        cprojT[:, k, :],
                        start=(k == 0),
                        stop=(k == KC - 1),
                    )
                if i == 0:
                    nc.vector.tensor_copy(mc_sb[:, i, :], pt)
                else:
                    nc.scalar.copy(mc_sb[:, i, :], pt)

            # tT[k] = sum_d c_projT[d,k] b_p[d]   as [NC, 1] (exp bias)
            t_ps = setup_ps.tile([NC, 1], f32, tag="setup_t")
            for k in range(KC):
                nc.tensor.matmul(
                    t_ps,
                    cprojT[:, k, :],
                    bp_sb[:, k : k + 1],
                    start=(k == 0),
                    stop=(k == KC - 1),
                )
            tT = const.tile([NC, 1], f32)
            nc.vector.tensor_copy(tT, t_ps)

            # cfstack [64, C+2] f32r = [fp22(CF) | 1 | 1]: one weighted-sum
            # matmul yields the weighted sum and the softmax normalizer.
            # (f32r matmuls need EVEN moving/dst free sizes, hence C+2.)
            cfstack = const.tile([NC, C + 2], f32r)
            nc.vector.tensor_copy(cfstack[:, :C], cf_sb)
            ones1 = const.tile([NC, 2], f32)
            nc.vector.memset(ones1, 1.0)
            nc.vector.tensor_copy(cfstack[:, C : C + 2], ones1)

            # ---------------- main loop --------------------------------------
            setup_stack.close()
            ps_stack = ExitStack()
            ps_xt = ps_stack.enter_context(
                tc.tile_pool(name="ps_xt", bufs=1, space="PSUM")
            )
            ps_sc = ps_stack.enter_context(
                tc.tile_pool(name="ps_sc", bufs=2, space="PSUM")
            )
            ps_ws = ps_stack.enter_context(
                tc.tile_pool(name="ps_ws", bufs=2, space="PSUM")
            )
            o_view = out.rearrange("(s p j) c -> s p j c", p=P, j=JCHUNK)

            def epilogue(st):
                """Weighted matmuls + normalize + residual for a finished
                half (one stage behind the scores pipeline)."""
                s, h, expT, x_tile, o_tile = st
                # two-chunk PSUM tiles (bank-pitched): one reciprocal covers
                # a pair of softmax normalizers at stride 512
                tmp = work.tile([P, HALF, C], f32, tag="tmp")
                for pair in range(HALF // 2):
                    ws2 = ps_ws.tile([P, 2, 512], f32, tag="ws", name="ws2")
                    for u in range(2):
                        jj = 2 * pair + u
                        nc.tensor.matmul(
                            ws2[:, u, : C + 2],
                            expT[:, bass.ts(jj, P)],
                            cfstack,
                            start=True,
                            stop=True,
                        )
                    recip2 = work.tile([P, 2], f32, tag="recip")
                    nc.vector.reciprocal(recip2, ws2[:, :, C])
                    for u in range(2):
                        jj = 2 * pair + u
                        if jj % 2 == 0:
                            nc.scalar.mul(
                                tmp[:, jj, :], ws2[:, u, :C], recip2[:, u : u + 1]
                            )
                        else:
                            nc.vector.tensor_scalar_mul(
                                tmp[:, jj, :], ws2[:, u, :C], recip2[:, u : u + 1]
                            )
                j0 = h * HALF
                tail = s >= NSUPER - 2
                if tail and h == 1:
                    # drain the GPSIMD backlog in parallel at the tail: DVE
                    # is idle once the last muls retire
                    nc.vector.tensor_add(
                        o_tile[:, j0 : j0 + HALF, :], tmp, x_tile[:, j0 : j0 + HALF, :]
                    )
                else:
                    nc.gpsimd.tensor_add(
                        o_tile[:, j0 : j0 + HALF, :], tmp, x_tile[:, j0 : j0 + HALF, :]
                    )
                # stores ride the GpSimd DGE queue: GPSIMD's adds are the
                # store's data dependency, so the wait is already resolved
                # when its sequencer reaches the dma_start (any other engine
                # would block its pipeline on the wait).  The tail stores go
                # on the idle Sync queue instead (all loads are done), and
                # the last supertile stores per half for a shorter drain.
                if s == NSUPER - 1:
                    nc.sync.dma_start(
                        o_view[s, :, h * HALF : (h + 1) * HALF],
                        o_tile[:, h * HALF : (h + 1) * HALF],
                    )
                elif h == 1:
                    if tail:
                        nc.sync.dma_start(o_view[s], o_tile)
                    else:
                        nc.gpsimd.dma_start(o_view[s], o_tile)

            prev = None
            for s in range(NSUPER):
                x_tile = x_tiles.pop(s)
                o_tile = oout.tile([P, JCHUNK, C], f32)

                for h in range(JCHUNK // HALF):
                    # X^T for 512 rows: per c-chunk k, [128, 512] (free =
                    # jj*128 + p  <->  row s*1024 + p*8 + (h*HALF+jj))
                    xt_ps = [
                        ps_xt.tile([P, RW], f32, tag=f"xt{k}", name=f"xt_ps{k}")
                        for k in range(KC)
                    ]
                    for jj in range(HALF):
                        j = h * HALF + jj
                        for k in range(KC):
                            nc.tensor.transpose(
                                xt_ps[k][:, bass.ts(jj, P)],
                                x_tile[:, j, bass.ts(k, P)],
                                ident,
                            )
                    xt_sb = [
                        work.tile([P, RW], f32r, tag=f"xt_sb{k}", name=f"xt_sb{k}")
                        for k in range(KC)
                    ]
                    # alternate the PSUM->SBUF copies between DVE and ACT
                    nc.vector.tensor_copy(xt_sb[0], xt_ps[0])
                    nc.scalar.copy(xt_sb[1], xt_ps[1])

                    # S^T[k, r] = sum_c M[c,k] X[r,c]
                    sc_ps = ps_sc.tile([NC, RW], f32, tag="sc")
                    for k in range(KC):
                        nc.tensor.matmul(
                            sc_ps,
                            mc_sb[:, k, :],
                            xt_sb[k],
                            start=(k == 0),
                            stop=(k == KC - 1),
                        )

                    # expT = exp(S^T + t)  (f32r: feeds the f32r matmul)
                    expT = work.tile([NC, RW], f32r, tag="expT")
                    nc.scalar.activation(expT, sc_ps, Exp, bias=tT)

                    if prev is not None:
                        epilogue(prev)
                        if h == 0 and s + 4 < NSUPER:
                            # rolling prefetch: the buffer being reused held
                            # x(s-1), whose last reader (epilogue (s-1,1))
                            # was just emitted above
                            load_x(s + 4)
                    prev = (s, h, expT, x_tile, o_tile)

            epilogue(prev)
            ps_stack.close()

    return _split_multiwait_ctrl(nc, mybir)


def _get_nc():
    if "nc" not in _CACHE:
        _CACHE["nc"] = _build()
    return _CACHE["nc"]


def run(inputs, trace=False):
    from concourse.bass_utils import run_bass_kernel_spmd

    nc = _get_nc()
    pf = np.ascontiguousarray(np.asarray(inputs["point_features"], dtype=np.float32))
    cfeat = np.ascontiguousarray(
        np.asarray(inputs["centroid_features"], dtype=np.float32)
    )
    wp = np.ascontiguousarray(np.asarray(inputs["W_p"], dtype=np.float32))
    bp = np.ascontiguousarray(np.asarray(inputs["b_p"], dtype=np.float32))
    wc = np.ascontiguousarray(np.asarray(inputs["W_c"], dtype=np.float32))
    bc = np.ascontiguousarray(np.asarray(inputs["b_c"], dtype=np.float32))

    in_maps = [
        {"x": pf[b], "cf": cfeat[b], "wp": wp, "bp": bp, "wc": wc, "bc": bc}
        for b in range(B)
    ]
    res = run_bass_kernel_spmd(nc, in_maps, core_ids=list(range(B)), trace=trace)
    out = np.stack([res.results[b]["out"] for b in range(B)], axis=0)
    return out, res


def kernel(**inputs) -> np.ndarray:
    out, _ = run(inputs, trace=False)
    return out


# revision 24
# speedup vs baseline: 1.5936x; 1.0448x over previous
"""Trainium2 Bass kernel for nn_AttentionFusion (8-core data-parallel over B).

Reference computation per batch b:
    p_proj = X @ W_p + b_p                      # (N, C)
    c_proj = CF @ W_c + b_c                     # (NC, C)
    S      = p_proj @ c_proj.T                  # (N, NC)
    W      = softmax(S, axis=-1)
    out    = X + W @ CF                         # (N, C)

Algebraic refactor (exact in real arithmetic):
    S = X @ M + 1·t  with  M = W_p @ c_proj.T (C×NC),  t = b_p @ c_proj.T (NC)
so the (N,C)x(C,C) projection matmul disappears; per-core PE work is the X
transposes (scores need C on partitions), the scores matmul, and the
weighted sum.

v2 changes vs the fp32 baseline (tolerance is 2e-2; fp22 everywhere in the
scores path costs ~2e-4):
  * transposes and scores matmuls run f32r (1.5 / 1.0 cyc per moving row
    instead of 2.0 / 4.0) -> PE drops from ~124us to ~75us busy.
  * no M duplication and no fp22-compensation rows: S^T/exp are [64, 512],
    the weighted matmul contracts K=64 against [fp22(CF) | 1].
  * PE is software-pipelined one half-tile: the weighted matmuls of half
    h-1 are emitted after the scores of half h so PE never stalls on exp.
  * epilogue: normalize alternates ACT/DVE into a [P,2,C] tmp, residual
    adds run on GPSIMD as two fused [128,512] tensor_tensor per half.
  * output is stored per half (512 rows, 4KiB/partition contiguous runs).

Sharding: B=8 batches -> one batch per NeuronCore, weights replicated.
"""

import numpy as np

B, N, NC, C = 8, 16384, 64, 256
P = 128  # SBUF partitions
SUPER_ROWS = 1024  # rows per DMA supertile (row = s*1024 + p*8 + j)
JCHUNK = SUPER_ROWS // P  # 8 row-chunks per supertile
HALF = 4  # chunks per scores tile (4*128 = 512 rows)
NSUPER = N // SUPER_ROWS

_CACHE = {}


def _split_multiwait_ctrl(nc, mybir):
    """This toolchain's walrus accepts only ONE sync wait per instruction,
    but Tile's scheduler attaches one wait per depended-on proc.  Keep the
    last wait on the instruction and hoist the excess onto single-wait NoOps
    inserted immediately before it on the same engine (same sequencer order,
    identical blocking semantics)."""
    for f in nc.m.functions:
        for bb in f.blocks:
            insts = bb.instructions
            new_list = []
            changed = False
            for inst in insts:
                si = inst.sync_info
                if si is not None and si.on_wait and len(si.on_wait) > 1:
                    waits = list(si.on_wait)
                    for w in waits[:-1]:
                        nop = mybir.InstNoOp(
                            name=nc.get_next_instruction_name(),
                            engine=inst.engine,
                            sync_info=mybir.SyncInfo(on_wait=[w], on_update=[]),
                            bass_nofuse=True,
                        )
                        nc.register_instruction(nop, overwrite=True)
                        new_list.append(nop)
                        changed = True
                    inst.sync_info = mybir.SyncInfo(
                        on_wait=[waits[-1]], on_update=list(si.on_update or [])
                    )
                new_list.append(inst)
            if changed:
                bb.instructions[:] = new_list
    return nc


def _build():
    from contextlib import ExitStack

    import concourse.bass as bass
    import concourse.mybir as mybir
    import concourse.tile as tile
    from concourse.masks import make_identity

    f32 = mybir.dt.float32
    f32r = mybir.dt.float32r
    Exp = mybir.ActivationFunctionType.Exp

    nc = bass.Bass("TRN2", target_bir_lowering=False, debug=False)
    x = nc.declare_dram_parameter("x", [N, C], f32, isOutput=False)
    cf = nc.declare_dram_parameter("cf", [NC, C], f32, isOutput=False)
    wp = nc.declare_dram_parameter("wp", [C, C], f32, isOutput=False)
    bp = nc.declare_dram_parameter("bp", [C], f32, isOutput=False)
    wc = nc.declare_dram_parameter("wc", [C, C], f32, isOutput=False)
    bc = nc.declare_dram_parameter("bc", [C], f32, isOutput=False)
    out = nc.declare_dram_parameter("out", [N, C], f32, isOutput=True)

    KC = C // P  # 2 contraction chunks of 128 over the C dim
    RW = HALF * P  # 512 rows per scores tile

    with tile.TileContext(nc) as tc:
        with (
            tc.tile_pool(name="const", bufs=1) as const,
            tc.tile_pool(name="xin", bufs=6) as xin,
            tc.tile_pool(name="oout", bufs=4) as oout,
            tc.tile_pool(name="work", bufs=5) as work,
        ):
            setup_stack = ExitStack()
            setup_ps = setup_stack.enter_context(
                tc.tile_pool(name="setup_ps", bufs=2, space="PSUM")
            )
            # ---------------- setup: identity, weights, M, t, cfstack ------
            x_view = x.rearrange("(s p j) c -> s p j c", p=P, j=JCHUNK)
            ident = const.tile([P, P], f32)
            make_identity(nc, ident)
            # dummy activation: hoists the ACT exp-table load off the
            # critical path (it otherwise fires right before the first exp)
            warm = const.tile([1, 2], f32)
            nc.vector.memset(warm, 0.0)
            nc.scalar.activation(warm, warm, Exp)

            # weight loads go FIRST on the Sync queue: the queue is in-order,
            # so the tiny weight transfers (0.6 MB) land before the x-load
            # flood and setup compute (which gates all in-order PE work)
            # finishes early
            cf_sb = const.tile([NC, C], f32)
            nc.sync.dma_start(cf_sb, cf.ap())
            bp_sb = const.tile([P, KC], f32)
            nc.sync.dma_start(bp_sb, bp.rearrange("(o p) -> p o", p=P))
            bc_sb = const.tile([P, KC], f32)
            nc.sync.dma_start(bc_sb, bc.rearrange("(o p) -> p o", p=P))
            wc_sb = const.tile([P, KC, C], f32)
            nc.sync.dma_start(wc_sb, wc.rearrange("(o p) d -> p o d", p=P))
            wp_sb = const.tile([P, KC, C], f32)
            nc.sync.dma_start(wp_sb, wp.rearrange("(o p) d -> p o d", p=P))

            # prime the input pipeline: 5 supertile loads queued up front
            # (6 bufs: the rolling load then reuses a buffer whose consumers
            # finished a full supertile earlier, so its trigger never waits)
            x_tiles = {}

            def load_x(sp):
                t = xin.tile([P, JCHUNK, C], f32, tag="x_tile", name="x_tile")
                nc.sync.dma_start(t, x_view[sp])
                x_tiles[sp] = t

            for sp in range(min(5, NSUPER)):
                load_x(sp)

            # cfT[c, k] = CF[k, c]   as [128, KC, NC]
            cfT = const.tile([P, KC, NC], f32)
            for i in range(KC):
                pt = setup_ps.tile([P, NC], f32, tag="setup")
                nc.tensor.transpose(pt, cf_sb[:, bass.ts(i, P)], ident[:NC, :NC])
                if i == 0:
                    nc.vector.tensor_copy(cfT[:, i, :], pt)
                else:
                    nc.scalar.copy(cfT[:, i, :], pt)

            # c_projT[d, k] = sum_c W_c[c,d] cfT[c,k] + b_c[d]   as [128, KC, NC]
            cprojT = const.tile([P, KC, NC], f32)
            for i in range(KC):
                pt = setup_ps.tile([P, NC], f32, tag="setup")
                for k in range(KC):
                    nc.tensor.matmul(
                        pt,
                        wc_sb[:, k, bass.ts(i, P)],
                        cfT[:, k, :],
                        start=(k == 0),
                        stop=(k == KC - 1),
                    )
                if i == 0:
                    nc.vector.tensor_scalar_add(
                        cprojT[:, i, :], pt, bc_sb[:, i : i + 1]
                    )
                else:
                    nc.scalar.add(cprojT[:, i, :], pt, bc_sb[:, i : i + 1])

            # wpT[d, c] = W_p[c, d]   as [128, KC, C]
            wpT = const.tile([P, KC, C], f32)
            for i in range(KC):  # d chunk
                pt = setup_ps.tile([P, KC, P], f32, tag="setupw")
                for j in range(KC):  # c chunk
                    nc.tensor.transpose(pt[:, j, :], wp_sb[:, j, bass.ts(i, P)], ident)
                if i == 0:
                    nc.vector.tensor_copy(wpT[:, i, :], pt)
                else:
                    nc.scalar.copy(wpT[:, i, :], pt)

            # M[c, k] = sum_d W_p[c,d] c_projT[d,k]  as f32r [128, KC, NC]
            mc_sb = const.tile([P, KC, NC], f32r)
            for i in range(KC):  # c chunk
                pt = setup_ps.tile([P, NC], f32, tag="setup")
                for k in range(KC):  # d chunk
                    nc.tensor.matmul(
                        pt,
                        wpT[:, k, bass.ts(i, P)],
                        cprojT[:, k, :],
                        start=(k == 0),
                        stop=(k == KC - 1),
                    )
                if i == 0:
                    nc.vector.tensor_copy(mc_sb[:, i, :], pt)
                else:
                    nc.scalar.copy(mc_sb[:, i, :], pt)

            # tT[k] = sum_d c_projT[d,k] b_p[d]   as [NC, 1] (exp bias)
            t_ps = setup_ps.tile([NC, 1], f32, tag="setup_t")
            for k in range(KC):
                nc.tensor.matmul(
                    t_ps,
                    cprojT[:, k, :],
                    bp_sb[:, k : k + 1],
                    start=(k == 0),
                    stop=(k == KC - 1),
                )
            tT = const.tile([NC, 1], f32)
            nc.vector.tensor_copy(tT, t_ps)

            # cfstack [64, C+2] f32r = [fp22(CF) | 1 | 1]: one weighted-sum
            # matmul yields the weighted sum and the softmax normalizer.
            # (f32r matmuls need EVEN moving/dst free sizes, hence C+2.)
            cfstack = const.tile([NC, C + 2], f32r)
            nc.vector.tensor_copy(cfstack[:, :C], cf_sb)
            ones1 = const.tile([NC, 2], f32)
            nc.vector.memset(ones1, 1.0)
            nc.vector.tensor_copy(cfstack[:, C : C + 2], ones1)

            # ---------------- main loop --------------------------------------
            setup_stack.close()
            ps_stack = ExitStack()
            ps_xt = ps_stack.enter_context(
                tc.tile_pool(name="ps_xt", bufs=1, space="PSUM")
            )
            ps_sc = ps_stack.enter_context(
                tc.tile_pool(name="ps_sc", bufs=2, space="PSUM")
            )
            ps_ws = ps_stack.enter_context(
                tc.tile_pool(name="ps_ws", bufs=4, space="PSUM")
            )
            o_view = out.rearrange("(s p j) c -> s p j c", p=P, j=JCHUNK)

            def epilogue(st):
                """Weighted matmuls + normalize + residual for a finished
                half (one stage behind the scores pipeline)."""
                s, h, expT, x_tile, o_tile = st
                tmp = work.tile([P, HALF, C], f32, tag="tmp")
                for jj in range(HALF):
                    # weighted[r, c|sum] = sum_k expT[k,r] [CF|1|1][k,c]
                    ws_ps = ps_ws.tile([P, C + 2], f32, tag="ws")
                    nc.tensor.matmul(
                        ws_ps,
                        expT[:, bass.ts(jj, P)],
                        cfstack,
                        start=True,
                        stop=True,
                    )
                    recip = work.tile([P, 1], f32, tag="recip")
                    nc.vector.reciprocal(recip, ws_ps[:, C : C + 1])
                    if jj % 2 == 0:
                        nc.scalar.mul(tmp[:, jj, :], ws_ps[:, :C], recip)
                    else:
                        nc.vector.tensor_scalar_mul(tmp[:, jj, :], ws_ps[:, :C], recip)
                j0 = h * HALF
                tail = s >= NSUPER - 2
                if tail and h == 1:
                    # drain the GPSIMD backlog in parallel at the tail: DVE
                    # is idle once the last muls retire
                    nc.vector.tensor_add(
                        o_tile[:, j0 : j0 + HALF, :], tmp, x_tile[:, j0 : j0 + HALF, :]
                    )
                else:
                    nc.gpsimd.tensor_add(
                        o_tile[:, j0 : j0 + HALF, :], tmp, x_tile[:, j0 : j0 + HALF, :]
                    )
                # stores ride the GpSimd DGE queue: GPSIMD's adds are the
                # store's data dependency, so the wait is already resolved
                # when its sequencer reaches the dma_start (any other engine
                # would block its pipeline on the wait).  The tail stores go
                # on the idle Sync queue instead (all loads are done), and
                # the last supertile stores per half for a shorter drain.
                if s == NSUPER - 1:
                    nc.sync.dma_start(
                        o_view[s, :, h * HALF : (h + 1) * HALF],
                        o_tile[:, h * HALF : (h + 1) * HALF],
                    )
                elif h == 1:
                    if tail:
                        nc.sync.dma_start(o_view[s], o_tile)
                    else:
                        nc.gpsimd.dma_start(o_view[s], o_tile)

            prev = None
            for s in range(NSUPER):
                x_tile = x_tiles.pop(s)
                o_tile = oout.tile([P, JCHUNK, C], f32)

                for h in range(JCHUNK // HALF):
                    # X^T for 512 rows: per c-chunk k, [128, 512] (free =
                    # jj*128 + p  <->  row s*1024 + p*8 + (h*HALF+jj))
                    xt_ps = [
                        ps_xt.tile([P, RW], f32, tag=f"xt{k}", name=f"xt_ps{k}")
                        for k in range(KC)
                    ]
                    for jj in range(HALF):
                        j = h * HALF + jj
                        for k in range(KC):
                            nc.tensor.transpose(
                                xt_ps[k][:, bass.ts(jj, P)],
                                x_tile[:, j, bass.ts(k, P)],
                                ident,
                            )
                    xt_sb = [
                        work.tile([P, RW], f32r, tag=f"xt_sb{k}", name=f"xt_sb{k}")
                        for k in range(KC)
                    ]
                    # alternate the PSUM->SBUF copies between DVE and ACT
                    nc.vector.tensor_copy(xt_sb[0], xt_ps[0])
                    nc.scalar.copy(xt_sb[1], xt_ps[1])

                    # S^T[k, r] = sum_c M[c,k] X[r,c]
                    sc_ps = ps_sc.tile([NC, RW], f32, tag="sc")
                    for k in range(KC):
                        nc.tensor.matmul(
                            sc_ps,
                            mc_sb[:, k, :],
                            xt_sb[k],
                            start=(k == 0),
                            stop=(k == KC - 1),
                        )

                    # expT = exp(S^T + t)  (f32r: feeds the f32r matmul)
                    expT = work.tile([NC, RW], f32r, tag="expT")
                    nc.scalar.activation(expT, sc_ps, Exp, bias=tT)

                    if prev is not None:
                        epilogue(prev)
                        if h == 0 and s + 4 < NSUPER:
                            # rolling prefetch: the buffer being reused held
                            # x(s-1), whose last reader (epilogue (s-1,1))
                            # was just emitted above
                            load_x(s + 4)
                    prev = (s, h, expT, x_tile, o_tile)

            epilogue(prev)
            ps_stack.close()

    return _split_multiwait_ctrl(nc, mybir)


def _get_nc():
    if "nc" not in _CACHE:
        _CACHE["nc"] = _build()
    return _CACHE["nc"]


def run(inputs, trace=False):
    from concourse.bass_utils import run_bass_kernel_spmd

    nc = _get_nc()
    pf = np.ascontiguousarray(np.asarray(inputs["point_features"], dtype=np.float32))
    cfeat = np.ascontiguousarray(
        np.asarray(inputs["centroid_features"], dtype=np.float32)
    )
    wp = np.ascontiguousarray(np.asarray(inputs["W_p"], dtype=np.float32))
    bp = np.ascontiguousarray(np.asarray(inputs["b_p"], dtype=np.float32))
    wc = np.ascontiguousarray(np.asarray(inputs["W_c"], dtype=np.float32))
    bc = np.ascontiguousarray(np.asarray(inputs["b_c"], dtype=np.float32))

    in_maps = [
        {"x": pf[b], "cf": cfeat[b], "wp": wp, "bp": bp, "wc": wc, "bc": bc}
        for b in range(B)
    ]
    res = run_bass_kernel_spmd(nc, in_maps, core_ids=list(range(B)), trace=trace)
    out = np.stack([res.results[b]["out"] for b in range(B)], axis=0)
    return out, res


def kernel(**inputs) -> np.ndarray:
    out, _ = run(inputs, trace=False)
    return out
